# revision 1
# baseline (speedup 1.0000x reference)
"""Trainium2 Bass kernel for the LDE1D vq_codebook problem.

Math (per batch b):
    q[t,k]   = 2*s0 * x[t,:] @ mu[k,:]          (PE, bf16 in / fp32 accum)
    p[t,k]   = exp(q[t,k])                      (ACT)
    pu[t,k]  = p[t,k] * u[k],  u = exp(-s0*||mu_k||^2)   (DVE ttr)
    D[t]     = sum_k pu[t,k]                    (same DVE ttr, accum_out)
    w[t,k]   = pu[t,k] * weights[t] / D[t]      (DVE)
    acc[k,:] = sum_t w[t,k] * [x[t,:], 1]       (PE, accumulated in PSUM)
    e[k,d]   = acc[k,d] / acc[k,D] - mu[k,d]
Softmax shift-invariance: the -s0*||x||^2 term is constant over k and drops.
The per-k factor u[k] cancels in acc[k,d]/acc[k,D], so using pu instead of
the exact softmax numerator is algebraically exact.

Sharding: data-parallel over B across 8 cores (8 batches each), mu/s
replicated. exp args are bounded (~|2*x.mu| <= ~20) so no max-subtract.
"""

import sys
from contextlib import ExitStack

import numpy as np

sys.path.insert(0, "/opt/trn_rl_repo")

import ml_dtypes

import concourse.bass as bass
import concourse.tile as tile
from concourse import bacc, mybir
from concourse.bass_utils import run_bass_kernel_spmd

BF16 = mybir.dt.bfloat16
F32 = mybir.dt.float32

B, T, D, K = 64, 4096, 256, 64
NCORES = 8
BPC = B // NCORES  # batches per core
TT = 128           # tokens per tile (partition dim)


def build_program(bpc=BPC, t=T, trn_type="TRN2"):
    ntiles = t // TT
    nc = bacc.Bacc(trn_type, target_bir_lowering=False, debug=False,
                   num_devices=NCORES)
    x_d = nc.dram_tensor("x", [bpc, t, D], F32, kind="ExternalInput").ap()
    wsT_d = nc.dram_tensor("wsT", [bpc, TT, ntiles], F32,
                           kind="ExternalInput").ap()
    muT2_d = nc.dram_tensor("muT2", [128, 2 * K], BF16,
                            kind="ExternalInput").ap()
    urep_d = nc.dram_tensor("urep", [128, K], BF16, kind="ExternalInput").ap()
    mu_d = nc.dram_tensor("mu", [K, D], F32, kind="ExternalInput").ap()
    ident_d = nc.dram_tensor("ident", [128, 128], BF16,
                             kind="ExternalInput").ap()
    out_d = nc.dram_tensor("out", [bpc, K, D], F32, kind="ExternalOutput").ap()

    with tile.TileContext(nc) as tc, ExitStack() as ctx:
        _body(ctx, tc, out_d, x_d, wsT_d, muT2_d, urep_d, mu_d, ident_d,
              bpc, ntiles)
    nc.compile()
    return nc


def _body(ctx, tc, out_d, x_d, wsT_d, muT2_d, urep_d, mu_d, ident_d,
          bpc, ntiles):
    nc = tc.nc
    const = ctx.enter_context(tc.tile_pool(name="const", bufs=1))
    muT2 = const.tile([128, 2 * K], BF16)
    nc.sync.dma_start(muT2[:], muT2_d[:])
    urep = const.tile([128, K], BF16)
    nc.sync.dma_start(urep[:], urep_d[:])
    mu_sb = const.tile([K, D], F32)
    nc.sync.dma_start(mu_sb[:], mu_d[:])
    ident = const.tile([128, 128], BF16)
    nc.sync.dma_start(ident[:], ident_d[:])

    xin_pool = ctx.enter_context(tc.tile_pool(name="xin", bufs=4))
    xbf_pool = ctx.enter_context(tc.tile_pool(name="xbf", bufs=4))
    xt_pool = ctx.enter_context(tc.tile_pool(name="xt", bufs=3))
    p_pool = ctx.enter_context(tc.tile_pool(name="p", bufs=3))
    pu_pool = ctx.enter_context(tc.tile_pool(name="pu", bufs=3))
    w_pool = ctx.enter_context(tc.tile_pool(name="w", bufs=3))
    sc_pool = ctx.enter_context(tc.tile_pool(name="sc", bufs=4))
    ws_pool = ctx.enter_context(tc.tile_pool(name="ws", bufs=2))
    res_pool = ctx.enter_context(tc.tile_pool(name="res", bufs=2))
    pt_psum = ctx.enter_context(tc.tile_pool(name="pt", bufs=2, space="PSUM"))
    pq_psum = ctx.enter_context(tc.tile_pool(name="pq", bufs=2, space="PSUM"))
    pe_psum = ctx.enter_context(tc.tile_pool(name="pe", bufs=2, space="PSUM"))

    for b in range(bpc):
        ws = ws_pool.tile([TT, ntiles], F32)
        nc.sync.dma_start(ws[:], wsT_d[b])
        acc = pe_psum.tile([K, D + 1], F32)
        for ti in range(ntiles):
            # load + cast
            xin = xin_pool.tile([TT, D], F32)
            nc.sync.dma_start(xin[:], x_d[b, ti * TT:(ti + 1) * TT, :])
            xbf = xbf_pool.tile([TT, D + 1], BF16)
            nc.gpsimd.tensor_copy(xbf[:, 0:D], xin[:])
            nc.gpsimd.memset(xbf[:, D:D + 1], 1.0)
            # transpose x tile (two 128x128 halves) via PE
            pt = pt_psum.tile([128, D], BF16)
            nc.tensor.transpose(pt[:, 0:128], xbf[:, 0:128], ident[:])
            nc.tensor.transpose(pt[:, 128:256], xbf[:, 128:256], ident[:])
            xt = xt_pool.tile([128, D], BF16)
            nc.scalar.copy(xt[:], pt[:])
            # q = x @ (2 s0 mu)^T : contract d in two halves
            pq = pq_psum.tile([TT, K], F32)
            nc.tensor.matmul(pq[:], xt[:, 0:128], muT2[:, 0:K],
                             start=True, stop=False)
            nc.tensor.matmul(pq[:], xt[:, 128:256], muT2[:, K:2 * K],
                             start=False, stop=True)
            # p = exp(q)  (bf16 out)
            p = p_pool.tile([TT, K], BF16)
            nc.scalar.activation(p[:], pq[:], mybir.ActivationFunctionType.Exp)
            # pu = p * u ;  D_t = sum_k pu
            pu = pu_pool.tile([TT, K], BF16)
            nc.vector.tensor_mul(pu[:], p[:], urep[:])
            dt = sc_pool.tile([TT, 1], F32, tag="dt")
            nc.vector.reduce_sum(dt[:], pu[:], axis=mybir.AxisListType.X)
            # scale_t = weights_t / D_t ; w = pu * scale_t
            rd = sc_pool.tile([TT, 1], F32, tag="rd")
            nc.vector.reciprocal(rd[:], dt[:])
            scl = sc_pool.tile([TT, 1], F32, tag="scl")
            nc.vector.tensor_tensor(scl[:], ws[:, ti:ti + 1], rd[:],
                                    mybir.AluOpType.mult)
            w = w_pool.tile([TT, K], BF16)
            nc.vector.tensor_scalar_mul(w[:], pu[:], scl[:])
            # acc[k, 0:D] += w^T x ; acc[k, D] += w^T 1
            nc.tensor.matmul(acc[:], w[:], xbf[:],
                             start=(ti == 0), stop=(ti == ntiles - 1))
        # epilogue: e = acc[:, :D]/acc[:, D] - mu
        rn = sc_pool.tile([K, 1], F32, tag="rn")
        nc.vector.reciprocal(rn[:], acc[:, D:D + 1])
        ex = res_pool.tile([K, D], F32, tag="ex")
        nc.vector.tensor_scalar_mul(ex[:], acc[:, 0:D], rn[:])
        res = res_pool.tile([K, D], F32, tag="res")
        nc.vector.tensor_sub(res[:], ex[:], mu_sb[:])
        nc.sync.dma_start(out_d[b], res[:])


def make_inputs(x, weights, mu, s, bpc=BPC, t=T):
    """Host-side prep: shard + precompute small replicated tensors."""
    ntiles = t // TT
    s = np.asarray(s, dtype=np.float32)
    s0 = float(s[0])
    if not np.allclose(s, s0):
        raise NotImplementedError("kernel assumes uniform s (as in setup)")
    mu = np.ascontiguousarray(mu, dtype=np.float32)
    mu2t = (2.0 * s0 * mu).T.astype(ml_dtypes.bfloat16)      # [D, K]
    muT2 = np.concatenate([mu2t[:128], mu2t[128:]], axis=1)  # [128, 2K]
    c = s0 * np.sum(mu.astype(np.float64) ** 2, axis=1)
    u = np.exp(-c).astype(ml_dtypes.bfloat16)                # [K]
    urep = np.broadcast_to(u, (128, K)).copy()
    ident = np.eye(128, dtype=ml_dtypes.bfloat16)
    ncores = x.shape[0] // bpc
    in_maps = []
    for ci in range(ncores):
        xs = np.ascontiguousarray(x[ci * bpc:(ci + 1) * bpc, :t],
                                  dtype=np.float32)
        wsl = weights[ci * bpc:(ci + 1) * bpc, :t].astype(np.float32)
        wsT = np.ascontiguousarray(
            wsl.reshape(bpc, ntiles, TT).transpose(0, 2, 1))  # [bpc,128,nt]
        in_maps.append({
            "x": xs, "wsT": wsT, "muT2": muT2, "urep": urep,
            "mu": mu, "ident": ident,
        })
    return in_maps


_CACHE = {}


def _get_program():
    if "nc" not in _CACHE:
        _CACHE["nc"] = build_program()
    return _CACHE["nc"]


def kernel(x, weights, mu, s):
    x = np.asarray(x)
    weights = np.asarray(weights)
    mu = np.asarray(mu, dtype=np.float32)
    s = np.asarray(s, dtype=np.float32)
    nc = _get_program()
    in_maps = make_inputs(x, weights, mu, s)
    res = run_bass_kernel_spmd(nc, in_maps, core_ids=list(range(NCORES)))
    outs = [res.results[ci]["out"].reshape(BPC, K * D)
            for ci in range(NCORES)]
    return np.concatenate(outs, axis=0).astype(np.float32)


if __name__ == "__main__":
    rng = np.random.default_rng(0)
    x = rng.standard_normal((B, T, D), dtype=np.float32)
    w = rng.random((B, T), dtype=np.float32)
    mu = (0.1 * rng.standard_normal((K, D))).astype(np.float32)
    s = np.ones((K,), dtype=np.float32)
    out = kernel(x, weights=w, mu=mu, s=s)
    print("out", out.shape, out.dtype)

